# revision 64
# baseline (speedup 1.0000x reference)
"""Causal multi-head attention mixer on 8 TRN2 NeuronCores.

Problem: x[2, 2048, 1024], 16 heads x 64 dim, RoPE, causal softmax, Wo proj.

Sharding (host-side): core c -> (batch b = c//4, head-group hg = c%4 of 4
heads = 256 channels). Each core computes its 4 heads' attention for its
batch and a partial output projection over its 256 Wo columns; the host
sums the 4 partials per batch (bf16 partials, f32 host sum).

v3 schedule: fully software-pipelined. xT is preloaded whole into SBUF.
QKV projection (8 seq-chunks of 256) and output projection are emitted as
"filler" work items interleaved into the attention ki-loops, so the PE
fills the exp-latency gaps; PV matmuls trail the score/exp stream by one
ki so the in-order PE queue never blocks on the scalar engine:

  proj(0); proj(1)
  attn(0) [filler: proj(2) | proj(3)]
  attn(1) [filler: proj(4)+normA(0) | normB(0)+proj(5)]
  attn(2) [filler: proj(6)+normA(1) | normB(1)+proj(7)]
  attn(3) [filler: norm(2)+outproj(0,1) | normA(3)+outproj(2)]
  normB(3); outproj(3)

PSUM (8 banks): scores double-buffer 2x[128,1024] (4 banks, merged heads
so exp ops stay wide); PV accumulators 2x[128,512] (tag "po"); everything
else (qk-proj, v-proj, out-proj, denominator broadcast) rotates through
2x[128,512] (tag "qp").

Engines: tensor = matmuls; scalar = exp only; vector = RoPE/mask/evict/
normalize; gpsimd = weight/swap/out DMA issue + memsets; sync = xT/swap
DMA issue. HW pitfalls baked in: custom DVE ops and partition_broadcast
ignore AP partition bases on HW (sim models them) -- all custom-DVE
inputs live in base-0 tiles; PSUM accumulation groups own a whole 2KB
bank (start zeroes the bank); walrus requires matmul fmap/weight at the
same SB partition base.
"""

import numpy as np
import ml_dtypes
from contextlib import ExitStack

import concourse.bass as bass
import concourse.tile as tile
from concourse import bacc, mybir
from concourse.bass_utils import run_bass_kernel_spmd

F32 = mybir.dt.float32
BF16 = mybir.dt.bfloat16
CDT = BF16
AOP = mybir.AluOpType
AF = mybir.ActivationFunctionType

S = 2048          # seq len
DM = 1024         # model dim
HPC = 4           # heads per core
DH = 64           # head dim
CH = HPC * DH     # channels per core = 256
PC = 256          # proj chunk (seq)
NPC = S // PC     # 8
QB = 512          # attention q-block
NB = S // QB      # 4
KT = 128          # k tile
NKT = S // KT     # 16
VE = DH + 1       # v columns per head incl. ones column = 65
ROPE_PERIOD = 10000.0


def _rope_tables():
    inv_freq = 1.0 / (ROPE_PERIOD ** (np.arange(0, DH, 2, dtype=np.float64) / DH))
    t = np.arange(S, dtype=np.float64)
    freqs = np.outer(inv_freq, t)           # [32, S]
    cos32 = np.cos(freqs)
    sin32 = np.sin(freqs)
    cos64 = np.concatenate([cos32, cos32], axis=0)          # [64, S]
    cosT = np.concatenate([cos64, cos64], axis=0)           # [128, S]
    # sin_signed rows 0:32 -> +sin (land on out[32:64] after swap),
    # rows 32:64 -> -sin (land on out[0:32]).
    sin64 = np.concatenate([sin32, -sin32], axis=0)         # [64, S]
    sinT2 = np.concatenate([sin64, sin64], axis=0)          # [128, S]
    return cosT, sinT2


def _build():
    nc = bacc.Bacc(None, target_bir_lowering=False)

    xT_ext = nc.dram_tensor("xT", [DM, S], CDT, kind="ExternalInput")
    wqkvT_ext = nc.dram_tensor("wqkvT", [DM, 3 * CH], CDT, kind="ExternalInput")
    woT_ext = nc.dram_tensor("woT", [CH, DM], CDT, kind="ExternalInput")
    out_ext = nc.dram_tensor("out", [S, DM], CDT, kind="ExternalOutput")

    cosT_np, sinT2_np = _rope_tables()

    # widened at PC granularity: per chunk c the [q|k] merged psum pair uses
    # cosW[:, 2*PC*c : 2*PC*(c+1)] = [cos(chunk) | cos(chunk)]
    def _widen(t):
        return np.concatenate(
            [np.concatenate([t[:, PC * c:PC * (c + 1)]] * 2, axis=1)
             for c in range(NPC)], axis=1).astype(ml_dtypes.bfloat16)
    cosW_dram = nc.inline_tensor(_widen(cosT_np), name="cosW")
    sinW_dram = nc.inline_tensor(_widen(sinT2_np), name="sinW")
    tri_np = np.where(np.arange(KT)[:, None] <= np.arange(KT)[None, :],
                      0.0, -1e9).astype(np.float32)
    tri_dram = nc.inline_tensor(tri_np, name="tri")
    onesr_np = np.ones((97, DH), dtype=ml_dtypes.bfloat16)
    onesr_dram = nc.inline_tensor(onesr_np, name="onesr")

    with tile.TileContext(nc) as tc, ExitStack() as ctx:
        const = ctx.enter_context(tc.tile_pool(name="const", bufs=1))
        persist = ctx.enter_context(tc.tile_pool(name="persist", bufs=1))

        cosW = const.tile([128, 2 * S], CDT, tag="cosW")
        sinW = const.tile([128, 2 * S], CDT, tag="sinW")
        tri = const.tile([KT, KT], F32, tag="tri")
        onesr = const.tile([97, DH], CDT, tag="onesr")

        xT_sb = [const.tile([128, S], CDT, name=f"xT{k}") for k in range(8)]
        wqkv_t = [const.tile([128, 3 * CH], CDT, name=f"wqkv{k}")
                  for k in range(8)]
        wq_t = [t[:, 0:CH] for t in wqkv_t]
        wk_t = [t[:, CH:2 * CH] for t in wqkv_t]
        wv_t = [t[:, 2 * CH:3 * CH] for t in wqkv_t]
        wo_t = [const.tile([128, DM], CDT, name=f"wo{k}") for k in range(2)]

        # -------- preload DMA issues (order matters: per-engine queues) ----
        # Startup is HBM-bound: land what chunks 0-1 (and attn(0)) need
        # first -- x cols 0:512 + Wq/Wk + the first table halves -- then
        # stream the rest with wide DMA lines.
        # sync is DMA-issue-rate-bound (~0.63us/issue): chunk-1's x rides the
        # near-empty scalar queue so attn(0) isn't gated by sync's backlog.
        for k in range(8):
            nc.sync.dma_start(xT_sb[k][:, 0:256], xT_ext[128 * k:128 * (k + 1), 0:256])
            nc.gpsimd.dma_start(wqkv_t[k][:, 0:2 * CH],
                                wqkvT_ext[128 * k:128 * (k + 1), 0:2 * CH])
        for k in range(8):
            nc.gpsimd.dma_start(wqkv_t[k][:, 2 * CH:3 * CH],
                                wqkvT_ext[128 * k:128 * (k + 1), 2 * CH:3 * CH])
        nc.scalar.dma_start(tri[:], tri_dram[:])
        nc.scalar.dma_start(onesr[:], onesr_dram[:])
        nc.scalar.dma_start(cosW[:, 0:2048], cosW_dram[:, 0:2048])
        nc.scalar.dma_start(sinW[:, 0:2048], sinW_dram[:, 0:2048])
        for k in range(8):
            nc.scalar.dma_start(xT_sb[k][:, 256:512], xT_ext[128 * k:128 * (k + 1), 256:512])
        for k in range(8):
            nc.sync.dma_start(xT_sb[k][:, 512:1024], xT_ext[128 * k:128 * (k + 1), 512:1024])
        for k in range(8):
            nc.sync.dma_start(xT_sb[k][:, 1024:2048], xT_ext[128 * k:128 * (k + 1), 1024:2048])
        nc.scalar.dma_start(cosW[:, 2048:4096], cosW_dram[:, 2048:4096])
        nc.scalar.dma_start(sinW[:, 2048:4096], sinW_dram[:, 2048:4096])
        for k in range(2):
            nc.scalar.dma_start(wo_t[k][:], woT_ext[128 * k:128 * (k + 1), :])

        # persistent activations
        qT_sb = [persist.tile([128, S], CDT, name=f"qT{m}") for m in range(2)]
        kp_sb = [persist.tile([128, S], CDT, name=f"kp{m}") for m in range(2)]
        attn_sb = [persist.tile([128, S], CDT, name=f"at{m}") for m in range(2)]
        v_sb = [persist.tile([128, HPC * VE], CDT, name=f"v{k}")
                for k in range(NKT)]
        oraw_sb = [persist.tile([DH, QB], F32, name=f"oraw{i}")
                   for i in range(16)]
        # softmax-denominator ones columns: set once, off the critical path
        for k in range(NKT):
            nc.gpsimd.memset(
                v_sb[k][:].rearrange("p (h e) -> p h e", h=HPC)[:, :, DH:VE],
                1.0)

        pp = ctx.enter_context(tc.tile_pool(name="pp", bufs=2, space="PSUM"))
        pop = ctx.enter_context(tc.tile_pool(name="pop", bufs=1, space="PSUM"))
        scp = ctx.enter_context(tc.tile_pool(name="scp", bufs=2, space="PSUM"))
        rpool = ctx.enter_context(tc.tile_pool(name="rpool", bufs=3))
        ptpool = ctx.enter_context(tc.tile_pool(name="ptpool", bufs=4))
        dpool = ctx.enter_context(tc.tile_pool(name="dpool", bufs=2))
        ospool = ctx.enter_context(tc.tile_pool(name="ospool", bufs=3))

        blk_state = {}

        # ---------------- work items ----------------
        def make_proj_items(c, ramp=False):
            cs = slice(PC * c, PC * (c + 1))
            st = {}

            def qk_item(kt):
                def f():
                    if kt == 0:
                        if ramp:
                            # chunks 0/1 predate any attention: borrow the
                            # idle score banks so chunk 1 doesn't WAR-wait
                            # on chunk 0's RoPE eviction.
                            big = scp.tile([128, 1024], F32, tag="sc",
                                           name=f"qkb_{c}")
                            st['qk'] = [big[:, 512 * m:512 * (m + 1)]
                                        for m in range(2)]
                        else:
                            st['qk'] = [pp.tile([128, 512], F32, tag="qp",
                                                name=f"qk{m}_{c}")
                                        for m in range(2)]
                    qk = st['qk']
                    xk = xT_sb[kt][:, cs]
                    for m in range(2):
                        nc.tensor.matmul(qk[m][:, 0:PC],
                                         wq_t[kt][:, 128 * m:128 * (m + 1)],
                                         xk, start=(kt == 0), stop=False)
                        nc.tensor.matmul(qk[m][:, PC:2 * PC],
                                         wk_t[kt][:, 128 * m:128 * (m + 1)],
                                         xk, start=False, stop=(kt == 7))
                return f

            def rope_item():
                def f():
                    qk = st.pop('qk')
                    ws = slice(2 * PC * c, 2 * PC * (c + 1))
                    t_t = rpool.tile([128, 1024], CDT, tag="ropet")
                    r_t = rpool.tile([128, 1024], CDT, tag="roper")
                    rs_t = rpool.tile([128, 1024], CDT, tag="ropes")
                    for m in range(2):
                        msl = slice(512 * m, 512 * (m + 1))
                        nc.vector.tensor_tensor(t_t[:, msl], qk[m][:],
                                                cosW[:, ws], AOP.mult)
                        nc.vector.tensor_tensor(r_t[:, msl], qk[m][:],
                                                sinW[:, ws], AOP.mult)
                    # 32-row partition swap as DVE copies: a DMA here would
                    # queue behind the whole preload backlog and stall the
                    # in-order vector pipe at the adds below.
                    for blk in range(4):
                        src = slice(32 * (blk ^ 1), 32 * (blk ^ 1) + 32)
                        dst_sl = slice(32 * blk, 32 * blk + 32)
                        nc.vector.tensor_copy(rs_t[dst_sl, :], r_t[src, :])
                    for m in range(2):
                        nc.vector.tensor_tensor(
                            qT_sb[m][:, cs], t_t[:, 512 * m:512 * m + 256],
                            rs_t[:, 512 * m:512 * m + 256], AOP.add)
                        nc.vector.tensor_tensor(
                            kp_sb[m][:, cs], t_t[:, 512 * m + 256:512 * m + 512],
                            rs_t[:, 512 * m + 256:512 * m + 512], AOP.add)
                return f

            def v_item(kt):
                def f():
                    if kt == 0:
                        st['v'] = pp.tile([128, 512], F32, tag="qp",
                                          name=f"v_{c}")
                    v_ps = st['v']
                    for sq in range(2):
                        nc.tensor.matmul(
                            v_ps[:, 256 * sq:256 * (sq + 1)],
                            xT_sb[kt][:, PC * c + 128 * sq:PC * c + 128 * (sq + 1)],
                            wv_t[kt][:], start=(kt == 0 and sq == 0),
                            stop=(kt == 7 and sq == 1))
                return f

            def vevict_item():
                def f():
                    v_ps = st.pop('v')
                    for sq in range(2):
                        vt = v_sb[2 * c + sq]
                        vt3 = vt[:].rearrange("p (h e) -> p h e", h=HPC)
                        nc.vector.tensor_copy(
                            vt3[:, :, 0:DH],
                            v_ps[:, 256 * sq:256 * (sq + 1)].rearrange(
                                "p (h d) -> p h d", h=HPC))
                return f

            return ([qk_item(k) for k in range(8)] + [rope_item()]
                    + [v_item(k) for k in range(8)] + [vevict_item()])

        def make_finish_items(b):
            qs = slice(QB * b, QB * (b + 1))
            st = {}

            def recip_item(m):
                # per-m denominator tile (base partition 0): custom DVE ops
                # mis-handle non-zero AP partition bases on HW, so each half
                # gets a full-tile reciprocal.
                def f():
                    dpack = blk_state[b][m]
                    dr32 = dpool.tile([33, QB], F32, tag=f"dr32_{m}",
                                      name=f"dr32_{m}_{b}")
                    drec = dpool.tile([33, QB], CDT, tag=f"drec_{m}",
                                      name=f"drec_{m}_{b}")
                    nc.vector.reciprocal_approx_fast(dr32[:], dpack[:])
                    nc.vector.tensor_copy(drec[:], dr32[:])
                    st[m] = drec
                return f

            def norm_item(r):
                def f():
                    m, hh = divmod(r, 2)
                    hrow = slice(64 * hh, 64 * hh + 64)
                    rbc = pp.tile([128, QB], F32, tag="qp", name=f"rbc{r}_{b}")
                    nc.tensor.matmul(rbc[0:DH, :], onesr[32 * hh:32 * hh + 1, :],
                                     st[m][32 * hh:32 * hh + 1, :],
                                     start=True, stop=True,
                                     tile_position=(32 * hh, 0))
                    nc.vector.tensor_tensor(
                        attn_sb[m][hrow, qs],
                        oraw_sb[4 * b + r][:], rbc[0:DH, :], AOP.mult)
                return f

            def outproj_item(sq):
                def f():
                    ssl = slice(128 * sq, 128 * (sq + 1))
                    ot = ospool.tile([128, DM], CDT, tag="ot")
                    for on in range(2):
                        osl = slice(512 * on, 512 * (on + 1))
                        ops = pp.tile([128, 512], F32, tag="qp",
                                      name=f"ops{sq}_{on}")
                        for ct in range(2):
                            nc.tensor.matmul(ops[:], attn_sb[ct][:, ssl],
                                             wo_t[ct][:, osl],
                                             start=(ct == 0), stop=(ct == 1))
                        nc.vector.tensor_copy(ot[:, osl], ops[:])
                        nc.gpsimd.dma_start(out_ext[ssl, osl], ot[:, osl])
                return f

            norm_a = [recip_item(0), norm_item(0), norm_item(1)]
            norm_b = [recip_item(1), norm_item(2), norm_item(3)]
            op_items = [outproj_item(sq) for sq in range(4 * b, 4 * b + 4)]
            return norm_a, norm_b, op_items

        def attn_block(b, filler_by_m):
            nkt = 4 * b + 4
            dpack_m = [dpool.tile([33, QB], F32, tag=f"dpack{m}",
                                  name=f"dpack{m}_{b}") for m in range(2)]
            for m in range(2):
                nc.gpsimd.memset(dpack_m[m][:], 1.0)
            blk_state[b] = dpack_m
            for m in range(2):
                filler = filler_by_m[m]
                per = (len(filler) + nkt - 1) // nkt if filler else 0
                po = [pop.tile([128, 512], F32, tag=f"po{hh}",
                               name=f"po{hh}_{m}_{b}") for hh in range(2)]
                # PV trails one ki behind the score/exp stream so the
                # in-order PE queue never blocks on exp(ki): while the
                # scalar engine computes exp(ki), the PE runs the score
                # matmuls of ki+1 (and fillers); PV(ki) then issues with
                # its input already available.
                def pv(ki, qlo):
                    for hh in range(2):
                        h = 2 * m + hh
                        nc.tensor.matmul(
                            po[hh][0:VE, qlo:QB],
                            v_sb[ki][:, VE * h:VE * (h + 1)],
                            pts[ki][:, QB * hh + qlo:QB * (hh + 1)],
                            start=(ki == 0), stop=(ki == nkt - 1))

                pts = {}
                for ki in range(nkt):
                    d = ki - 4 * b
                    qlo = max(0, 128 * d)
                    s_t = scp.tile([128, 1024], F32, tag="sc",
                                   name=f"sc_{m}_{b}_{ki}")
                    sc3 = s_t[:].rearrange("p (g q) -> p g q", g=2)
                    for hh in range(2):
                        hsl = slice(64 * hh, 64 * (hh + 1))
                        nc.tensor.matmul(
                            sc3[:, hh, qlo:QB],
                            kp_sb[m][hsl, 128 * ki:128 * (ki + 1)],
                            qT_sb[m][hsl, QB * b + qlo:QB * (b + 1)],
                            start=True, stop=True)
                    if d >= 0:
                        nc.vector.tensor_tensor(
                            sc3[:, :, qlo:qlo + 128],
                            sc3[:, :, qlo:qlo + 128],
                            tri[:].unsqueeze(1).broadcast_to([128, 2, 128]),
                            AOP.add)
                    p_t = ptpool.tile([128, 1024], CDT, tag="pt",
                                      name=f"pt_{m}_{b}_{ki}")
                    pts[ki] = p_t
                    pt3 = p_t[:].rearrange("p (g q) -> p g q", g=2)
                    nc.scalar.activation(pt3[:, :, qlo:QB], sc3[:, :, qlo:QB],
                                         AF.Exp, scale=0.125)
                    if ki > 0:
                        pv(ki - 1, max(0, 128 * (ki - 1 - 4 * b)))
                    for _ in range(per):
                        if filler:
                            filler.pop(0)()
                pv(nkt - 1, max(0, 128 * (nkt - 1 - 4 * b)))
                for hh in range(2):
                    r = 2 * m + hh
                    nc.vector.tensor_copy(oraw_sb[4 * b + r][:],
                                          po[hh][0:DH, :])
                    nc.vector.tensor_copy(dpack_m[m][32 * hh:32 * hh + 1, :],
                                          po[hh][DH:DH + 1, :])
                while filler:
                    filler.pop(0)()

        # ---------------- schedule ----------------
        # Normalization of block b-1 runs inside attn(b); the out-projections
        # of blocks 0-2 are held back to fill attn(3), the longest exp-paced
        # stretch, which otherwise has no tensor-engine work left. The last
        # block's own normalization for m=0 overlaps its m=1 ki-loop.
        for it in make_proj_items(0, ramp=True):
            it()
        for it in make_proj_items(1, ramp=True):
            it()
        op_bank = []
        for b in range(NB):
            if b > 0:
                norm_a, norm_b, op_items = make_finish_items(b - 1)
                op_bank.append(op_items)
            if b == 0:
                f0 = make_proj_items(2)
                f1 = make_proj_items(3)
            elif b < NB - 1:
                f0 = make_proj_items(2 * b + 2) + norm_a
                f1 = norm_b + make_proj_items(2 * b + 3)
            else:
                f0 = norm_a + norm_b + op_bank[0] + op_bank[1]
                last_a, last_b, last_ops = make_finish_items(b)
                f1 = last_a + op_bank[2]
            attn_block(b, (f0, f1))
        for it in last_b + last_ops:
            it()

    nc.compile()
    return nc


_NC_CACHE = []


def kernel(x, Wq, Wk, Wv, Wo):
    x = np.asarray(x, dtype=np.float32)
    Wq = np.asarray(Wq, dtype=np.float32)
    Wk = np.asarray(Wk, dtype=np.float32)
    Wv = np.asarray(Wv, dtype=np.float32)
    Wo = np.asarray(Wo, dtype=np.float32)

    np_cdt = ml_dtypes.bfloat16
    in_maps = []
    for c in range(8):
        b, hg = divmod(c, 4)
        rows = slice(CH * hg, CH * (hg + 1))
        wqkv = np.concatenate(
            [Wq[rows, :].T, Wk[rows, :].T, Wv[rows, :].T], axis=1)
        in_maps.append({
            "xT": np.ascontiguousarray(x[b].T).astype(np_cdt),
            "wqkvT": np.ascontiguousarray(wqkv).astype(np_cdt),
            "woT": np.ascontiguousarray(Wo[:, rows].T).astype(np_cdt),
        })

    if not _NC_CACHE:
        _NC_CACHE.append(_build())
    nc = _NC_CACHE[0]

    res = run_bass_kernel_spmd(nc, in_maps, list(range(8)))
    out = np.zeros((2, S, DM), dtype=np.float32)
    for c in range(8):
        out[c // 4] += res.results[c]["out"].astype(np.float32)
    return out
